# revision 11
# baseline (speedup 1.0000x reference)
"""L1 pairwise distance kernel for Trainium2, 8 NeuronCores.

res[i, j] = sum_d |x1c[i, d] - x2c[j, d]|,  x1c/x2c centered by mean(x1).

Per core: 256 x1 rows (data-parallel over rows), 2 halves of 128 rows =
64 row-pairs each. Every engine produces absdiff tiles for a share of
the pairs, sized so all four engines finish together:

  A-path (DVE, NA pairs/half): fp16 tile [128,2048] = |x2c^T - x1|
    via tensor_scalar add (4x mode) + batched int16 AND (abs); fp16
    one-hot mask matmuls accumulate into PSUM_A (1 col/cycle).
  J-path (ACT, NJ pairs/half): activation Abs emits the tile directly
    in fp8e4 (ACT cost is dtype-independent). Tile pairs feed DoubleRow
    fp8 matmuls (256-deep contraction, same 216ns/512-col chunk but 2
    pairs per chunk) into PSUM_J.
  P-path (Pool, NP pairs/half): gpsimd fused (add bias, max 0) -> R and
    (add bias, min 0) -> M in fp8; |d| = R - M via DoubleRow with +1/-1
    masks, accumulated into PSUM_J.

  fp8 rows carry ~1.4e-2 rel err (fp8e4 quantization), within the 2e-2
  gate. PSUM_A copy lands on DVE, PSUM_J copy on ACT; two DMAs per half
  write the output rows.

Self-contained: hardcodes shapes from the problem spec.
"""

import numpy as np
import ml_dtypes

import bass_rust
import concourse.bass as bass
import concourse.tile as tile
from concourse import mybir
import concourse.bass_utils as bu

N1 = 2048
N2 = 2048
D = 64
NCORES = 8
IPC = N1 // NCORES          # 256 x1 rows per core
NPAIR_HALF = 64
JCH = 512                   # matmul free-dim chunk (one PSUM bank)
NA = 34                     # DVE pairs per half
NJ = 22                     # ACT fp8 pairs per half (even)
NP = NPAIR_HALF - NA - NJ   # Pool fp8 pairs per half
NGJ = NJ // 2               # DoubleRow groups (2 pairs each)
F32 = mybir.dt.float32
F16 = mybir.dt.float16
F8 = mybir.dt.float8e4
I16 = mybir.dt.int16
A = mybir.AluOpType
DR = mybir.MatmulPerfMode.DoubleRow
ABS = mybir.ActivationFunctionType.Abs

_nop_counter = [0]


def _split_multi_waits(nc):
    """This container's walrus build allows one sync-wait per instruction.
    Move extra waits onto same-engine NoOps placed just before."""
    for fn in nc.m.functions:
        for blk in fn.blocks:
            out = []
            changed = False
            for inst in blk.instructions:
                si = inst.sync_info
                if si is not None and len(si.on_wait) > 1:
                    waits = list(si.on_wait)
                    for w in waits[:-1]:
                        _nop_counter[0] += 1
                        nop = mybir.InstNoOp(
                            name=f"I-waitsplit-{_nop_counter[0]}", ins=[], outs=[]
                        )
                        nop.engine = inst.engine
                        nop.sync_info = bass_rust.SyncInfo(on_wait=[w], on_update=[])
                        if inst.debug is not None:
                            nop.debug = inst.debug
                        out.append(nop)
                        nc.register_instruction(nop, overwrite=True)
                    si.on_wait = waits[-1:]
                    changed = True
                out.append(inst)
            if changed:
                blk.instructions = out


def _schedule(counts):
    """Proportional merge: yields (kind, idx) interleaved by fraction."""
    items = []
    for kind, n in counts.items():
        for i in range(n):
            items.append(((i + 0.5) / n, kind, i))
    items.sort()
    return [(k, i) for _, k, i in items]


def _build():
    nc = bass.Bass()
    x2s16_d = nc.dram_tensor("x2s16", [128, N2], F16, kind="ExternalInput")
    bias_d = nc.dram_tensor("bias", [128, IPC // 2], F32, kind="ExternalInput")
    maskb16_d = nc.dram_tensor("maskb16", [128, 254], F16, kind="ExternalInput")
    jmask8_d = nc.dram_tensor("jmask8", [128, NGJ, 2, 128], F8, kind="ExternalInput")
    pmask8_d = nc.dram_tensor("pmask8", [128, NP, 2, 128], F8, kind="ExternalInput")
    out_d = nc.dram_tensor("out", [IPC, N2], F32, kind="ExternalOutput")

    with tile.TileContext(nc) as tc:
        with (
            tc.tile_pool(name="singles", bufs=1) as singles,
            tc.tile_pool(name="ad", bufs=4) as adpool,
            tc.tile_pool(name="jd", bufs=3) as jdpool,
            tc.tile_pool(name="rm", bufs=3) as rmpool,
            tc.tile_pool(name="psa", bufs=1, space="PSUM") as psapool,
            tc.tile_pool(name="psj", bufs=1, space="PSUM") as psjpool,
            tc.tile_pool(name="ob", bufs=4) as outpool,
        ):
            x2s16 = singles.tile([128, N2], F16)
            nc.sync.dma_start(x2s16[:], x2s16_d[:])
            bias = singles.tile([128, IPC // 2], F32)
            nc.sync.dma_start(bias[:], bias_d[:])
            maskb16 = singles.tile([128, 254], F16)
            nc.sync.dma_start(maskb16[:], maskb16_d[:])
            jmask8 = singles.tile([128, NGJ, 2, 128], F8)
            nc.sync.dma_start(jmask8[:], jmask8_d[:])
            pmask8 = singles.tile([128, NP, 2, 128], F8)
            nc.sync.dma_start(pmask8[:], pmask8_d[:])
            andmask = singles.tile([128, 1], I16)
            nc.vector.memset(andmask[:], 0x7FFF)

            nab = (NA + 1) // 2  # A-blocks of 2 pairs (batched AND)
            for h in range(2):
                psA = psapool.tile([128, N2], F32)
                psJ = psjpool.tile([128, N2], F32)
                nfp8 = NGJ + NP  # fp8 accumulation groups in psJ
                fp8_done = 0
                for kind, idx in _schedule({"a": nab, "j": NGJ, "p": NP}):
                    if kind == "a":
                        # A-block: 2 pairs (or 1 tail pair), batched AND
                        blkn = min(2, NA - 2 * idx)
                        ad = adpool.tile([128, 2, N2], F16, tag="ad")
                        for i in range(blkn):
                            a = 2 * idx + i
                            pi = h * NPAIR_HALF + a
                            nc.vector.tensor_scalar(
                                out=ad[:, i, :], in0=x2s16[:],
                                scalar1=bias[:, pi : pi + 1], scalar2=None,
                                op0=A.add,
                            )
                        flat = ad[:, 0 : blkn, :].bitcast(I16)
                        nc.vector.tensor_scalar(
                            out=flat, in0=flat,
                            scalar1=andmask[:], scalar2=None,
                            op0=A.bitwise_and,
                        )
                        for i in range(blkn):
                            a = 2 * idx + i
                            mg = maskb16[:, 126 - 2 * a : 254 - 2 * a]
                            for jc in range(N2 // JCH):
                                nc.tensor.matmul(
                                    psA[:, jc * JCH : (jc + 1) * JCH],
                                    mg,
                                    ad[:, i, jc * JCH : (jc + 1) * JCH],
                                    start=(a == 0),
                                    stop=(a == NA - 1),
                                )
                    elif kind == "j":
                        g = idx
                        jt = jdpool.tile([128, 2, N2], F8, tag="jt")
                        for i in range(2):
                            pi = h * NPAIR_HALF + NA + 2 * g + i
                            nc.scalar.activation(
                                out=jt[:, i, :], in_=x2s16[:],
                                func=ABS,
                                bias=bias[:, pi : pi + 1], scale=1.0,
                            )
                        for jc in range(N2 // JCH):
                            nc.tensor.matmul(
                                psJ[:, jc * JCH : (jc + 1) * JCH],
                                jmask8[:, g, :, :],
                                jt[:, :, jc * JCH : (jc + 1) * JCH],
                                start=(fp8_done == 0),
                                stop=(fp8_done == nfp8 - 1),
                                perf_mode=DR,
                            )
                        fp8_done += 1
                    else:
                        p = idx
                        pi = h * NPAIR_HALF + NA + NJ + p
                        rm = rmpool.tile([128, 2, N2], F8, tag="rm")
                        nc.gpsimd.tensor_scalar(
                            out=rm[:, 0, :], in0=x2s16[:],
                            scalar1=bias[:, pi : pi + 1], scalar2=0.0,
                            op0=A.add, op1=A.max,
                        )
                        nc.gpsimd.tensor_scalar(
                            out=rm[:, 1, :], in0=x2s16[:],
                            scalar1=bias[:, pi : pi + 1], scalar2=0.0,
                            op0=A.add, op1=A.min,
                        )
                        for jc in range(N2 // JCH):
                            nc.tensor.matmul(
                                psJ[:, jc * JCH : (jc + 1) * JCH],
                                pmask8[:, p, :, :],
                                rm[:, :, jc * JCH : (jc + 1) * JCH],
                                start=(fp8_done == 0),
                                stop=(fp8_done == nfp8 - 1),
                                perf_mode=DR,
                            )
                        fp8_done += 1

                obA = outpool.tile([128, N2], F32, tag="obA")
                obJ = outpool.tile([128, N2], F32, tag="obJ")
                nc.vector.tensor_copy(obA[:], psA[:])
                nc.scalar.copy(obJ[:], psJ[:])
                nc.sync.dma_start(
                    out_d[h * 128 : h * 128 + 2 * NA, :], obA[0 : 2 * NA, :]
                )
                nc.sync.dma_start(
                    out_d[h * 128 + 2 * NA : (h + 1) * 128, :], obJ[2 * NA : 128, :]
                )
    _split_multi_waits(nc)
    return nc


_cached_nc = None


def _prep_inputs(x1, x2):
    x1 = np.asarray(x1, dtype=np.float32)
    x2 = np.asarray(x2, dtype=np.float32)
    adj = x1.mean(axis=0, dtype=np.float32).astype(np.float32)
    x1c = x1 - adj
    x2c = x2 - adj

    x2s16 = np.tile(np.ascontiguousarray(x2c.T), (2, 1)).astype(np.float16)

    maskb = np.zeros((128, 254), dtype=np.float32)
    k = np.arange(128)
    maskb[k, 126 + k // 64] = 1.0
    maskb16 = maskb.astype(np.float16)

    # J: group g covers pairs NA+2g, NA+2g+1 -> partitions 2NA+4g .. +3
    jmask8 = np.zeros((128, NGJ, 2, 128), dtype=ml_dtypes.float8_e4m3fn)
    for g in range(NGJ):
        for i in range(2):
            jmask8[k, g, i, 2 * NA + 4 * g + 2 * i + k // 64] = 1.0
    # P: pair p covers pair NA+NJ+p -> partitions 2(NA+NJ+p), +1; R:+1, M:-1
    pmask8 = np.zeros((128, NP, 2, 128), dtype=ml_dtypes.float8_e4m3fn)
    for p in range(NP):
        m0 = 2 * (NA + NJ + p)
        pmask8[k, p, 0, m0 + k // 64] = 1.0
        pmask8[k, p, 1, m0 + k // 64] = -1.0

    in_maps = []
    for c in range(NCORES):
        sl = x1c[c * IPC : (c + 1) * IPC]          # [256, 64]
        b = -np.transpose(sl.reshape(IPC // 2, 2, D), (1, 2, 0)).reshape(128, IPC // 2)
        in_maps.append({
            "x2s16": x2s16,
            "bias": np.ascontiguousarray(b, dtype=np.float32),
            "maskb16": maskb16,
            "jmask8": jmask8.view(np.uint8),
            "pmask8": pmask8.view(np.uint8),
        })
    return in_maps


def run(x1, x2, trace=False):
    global _cached_nc
    if _cached_nc is None:
        _cached_nc = _build()
    in_maps = _prep_inputs(x1, x2)
    r = bu.run_bass_kernel_spmd(
        _cached_nc, in_maps, core_ids=list(range(NCORES)), trace=trace
    )
    out = np.concatenate([r.results[c]["out"] for c in range(NCORES)], axis=0)
    return out, r


def kernel(x1, x2):
    out, _ = run(x1, x2, trace=False)
    return out


# revision 12
# speedup vs baseline: 8.2653x; 8.2653x over previous
"""L1 pairwise distance kernel for Trainium2, 8 NeuronCores.

res[i, j] = sum_d |x1c[i, d] - x2c[j, d]|,  x1c/x2c centered by mean(x1).

Per core: 256 x1 rows (data-parallel over rows), 2 halves of 128 rows =
64 row-pairs each. Every engine produces absdiff tiles for a share of
the pairs, sized so all four engines finish together:

  A-path (DVE, NA pairs/half): fp16 tile [128,2048] = |x2c^T - x1|
    via tensor_scalar add (4x mode) + batched int16 AND (abs); fp16
    one-hot mask matmuls accumulate into PSUM_A (1 col/cycle).
  J-path (ACT, NJ pairs/half): activation Abs emits the tile directly
    in fp8e4 (ACT cost is dtype-independent). Tile pairs feed DoubleRow
    fp8 matmuls (256-deep contraction, same 216ns/512-col chunk but 2
    pairs per chunk) into PSUM_J.
  P-path (Pool, NP pairs/half): gpsimd fused (add bias, max 0) -> R and
    (add bias, min 0) -> M in fp8; |d| = R - M via DoubleRow with +1/-1
    masks, accumulated into PSUM_J.

  fp8 rows carry ~1.4e-2 rel err (fp8e4 quantization), within the 2e-2
  gate. PSUM_A copy lands on DVE, PSUM_J copy on ACT; two DMAs per half
  write the output rows.

Self-contained: hardcodes shapes from the problem spec.
"""

import numpy as np
import ml_dtypes

import bass_rust
import concourse.bass as bass
import concourse.tile as tile
from concourse import mybir
import concourse.bass_utils as bu

N1 = 2048
N2 = 2048
D = 64
NCORES = 8
IPC = N1 // NCORES          # 256 x1 rows per core
NPAIR_HALF = 64
JCH = 512                   # matmul free-dim chunk (one PSUM bank)
NA = 38                     # DVE pairs per half
NJ = 26                     # ACT fp8 pairs per half (even)
NP = NPAIR_HALF - NA - NJ   # Pool fp8 pairs per half
NGJ = NJ // 2               # DoubleRow groups (2 pairs each)
F32 = mybir.dt.float32
F16 = mybir.dt.float16
F8 = mybir.dt.float8e4
I16 = mybir.dt.int16
A = mybir.AluOpType
DR = mybir.MatmulPerfMode.DoubleRow
ABS = mybir.ActivationFunctionType.Abs

_nop_counter = [0]


def _split_multi_waits(nc):
    """This container's walrus build allows one sync-wait per instruction.
    Move extra waits onto same-engine NoOps placed just before."""
    for fn in nc.m.functions:
        for blk in fn.blocks:
            out = []
            changed = False
            for inst in blk.instructions:
                si = inst.sync_info
                if si is not None and len(si.on_wait) > 1:
                    waits = list(si.on_wait)
                    for w in waits[:-1]:
                        _nop_counter[0] += 1
                        nop = mybir.InstNoOp(
                            name=f"I-waitsplit-{_nop_counter[0]}", ins=[], outs=[]
                        )
                        nop.engine = inst.engine
                        nop.sync_info = bass_rust.SyncInfo(on_wait=[w], on_update=[])
                        if inst.debug is not None:
                            nop.debug = inst.debug
                        out.append(nop)
                        nc.register_instruction(nop, overwrite=True)
                    si.on_wait = waits[-1:]
                    changed = True
                out.append(inst)
            if changed:
                blk.instructions = out


def _schedule(counts):
    """Proportional merge: yields (kind, idx) interleaved by fraction."""
    items = []
    for kind, n in counts.items():
        for i in range(n):
            items.append(((i + 0.5) / n, kind, i))
    items.sort()
    return [(k, i) for _, k, i in items]


def _build():
    nc = bass.Bass()
    x2s16_d = nc.dram_tensor("x2s16", [128, N2], F16, kind="ExternalInput")
    bias_d = nc.dram_tensor("bias", [128, IPC // 2], F32, kind="ExternalInput")
    maskb16_d = nc.dram_tensor("maskb16", [128, 254], F16, kind="ExternalInput")
    jmask8_d = nc.dram_tensor("jmask8", [128, NGJ, 2, 128], F8, kind="ExternalInput")
    out_d = nc.dram_tensor("out", [IPC, N2], F32, kind="ExternalOutput")

    with tile.TileContext(nc) as tc:
        with (
            tc.tile_pool(name="singles", bufs=1) as singles,
            tc.tile_pool(name="ad", bufs=4) as adpool,
            tc.tile_pool(name="jd", bufs=3) as jdpool,
            tc.tile_pool(name="psa", bufs=1, space="PSUM") as psapool,
            tc.tile_pool(name="psj", bufs=1, space="PSUM") as psjpool,
            tc.tile_pool(name="ob", bufs=4) as outpool,
        ):
            x2s16 = singles.tile([128, N2], F16)
            nc.sync.dma_start(x2s16[:], x2s16_d[:])
            bias = singles.tile([128, IPC // 2], F32)
            nc.sync.dma_start(bias[:], bias_d[:])
            maskb16 = singles.tile([128, 254], F16)
            nc.sync.dma_start(maskb16[:], maskb16_d[:])
            jmask8 = singles.tile([128, NGJ, 2, 128], F8)
            nc.sync.dma_start(jmask8[:], jmask8_d[:])
            andmask = singles.tile([128, 1], I16)
            nc.vector.memset(andmask[:], 0x7FFF)

            nab = (NA + 1) // 2  # A-blocks of 2 pairs (batched AND)
            for h in range(2):
                psA = psapool.tile([128, N2], F32)
                psJ = psjpool.tile([128, N2], F32)
                nfp8 = NGJ  # fp8 accumulation groups in psJ
                fp8_done = 0
                for kind, idx in _schedule({"a": nab, "j": NGJ}):
                    if kind == "a":
                        # A-block: 2 pairs (or 1 tail pair), batched AND
                        blkn = min(2, NA - 2 * idx)
                        ad = adpool.tile([128, 2, N2], F16, tag="ad")
                        for i in range(blkn):
                            a = 2 * idx + i
                            pi = h * NPAIR_HALF + a
                            nc.vector.tensor_scalar(
                                out=ad[:, i, :], in0=x2s16[:],
                                scalar1=bias[:, pi : pi + 1], scalar2=None,
                                op0=A.add,
                            )
                        flat = ad[:, 0 : blkn, :].bitcast(I16)
                        nc.vector.tensor_scalar(
                            out=flat, in0=flat,
                            scalar1=andmask[:], scalar2=None,
                            op0=A.bitwise_and,
                        )
                        for i in range(blkn):
                            a = 2 * idx + i
                            mg = maskb16[:, 126 - 2 * a : 254 - 2 * a]
                            for jc in range(N2 // JCH):
                                nc.tensor.matmul(
                                    psA[:, jc * JCH : (jc + 1) * JCH],
                                    mg,
                                    ad[:, i, jc * JCH : (jc + 1) * JCH],
                                    start=(a == 0),
                                    stop=(a == NA - 1),
                                )
                    elif kind == "j":
                        g = idx
                        jt = jdpool.tile([128, 2, N2], F8, tag="jt")
                        for i in range(2):
                            pi = h * NPAIR_HALF + NA + 2 * g + i
                            nc.scalar.activation(
                                out=jt[:, i, :], in_=x2s16[:],
                                func=ABS,
                                bias=bias[:, pi : pi + 1], scale=1.0,
                            )
                        for jc in range(N2 // JCH):
                            nc.tensor.matmul(
                                psJ[:, jc * JCH : (jc + 1) * JCH],
                                jmask8[:, g, :, :],
                                jt[:, :, jc * JCH : (jc + 1) * JCH],
                                start=(fp8_done == 0),
                                stop=(fp8_done == nfp8 - 1),
                                perf_mode=DR,
                            )
                        fp8_done += 1
                obA = outpool.tile([128, N2], F32, tag="obA")
                obJ = outpool.tile([128, N2], F32, tag="obJ")
                nc.vector.tensor_copy(obA[:], psA[:])
                nc.scalar.copy(obJ[:], psJ[:])
                nc.sync.dma_start(
                    out_d[h * 128 : h * 128 + 2 * NA, :], obA[0 : 2 * NA, :]
                )
                nc.sync.dma_start(
                    out_d[h * 128 + 2 * NA : (h + 1) * 128, :], obJ[2 * NA : 128, :]
                )
    _split_multi_waits(nc)
    return nc


_cached_nc = None


def _prep_inputs(x1, x2):
    x1 = np.asarray(x1, dtype=np.float32)
    x2 = np.asarray(x2, dtype=np.float32)
    adj = x1.mean(axis=0, dtype=np.float32).astype(np.float32)
    x1c = x1 - adj
    x2c = x2 - adj

    x2s16 = np.tile(np.ascontiguousarray(x2c.T), (2, 1)).astype(np.float16)

    maskb = np.zeros((128, 254), dtype=np.float32)
    k = np.arange(128)
    maskb[k, 126 + k // 64] = 1.0
    maskb16 = maskb.astype(np.float16)

    # J: group g covers pairs NA+2g, NA+2g+1 -> partitions 2NA+4g .. +3
    jmask8 = np.zeros((128, NGJ, 2, 128), dtype=ml_dtypes.float8_e4m3fn)
    for g in range(NGJ):
        for i in range(2):
            jmask8[k, g, i, 2 * NA + 4 * g + 2 * i + k // 64] = 1.0

    in_maps = []
    for c in range(NCORES):
        sl = x1c[c * IPC : (c + 1) * IPC]          # [256, 64]
        b = -np.transpose(sl.reshape(IPC // 2, 2, D), (1, 2, 0)).reshape(128, IPC // 2)
        in_maps.append({
            "x2s16": x2s16,
            "bias": np.ascontiguousarray(b, dtype=np.float32),
            "maskb16": maskb16,
            "jmask8": jmask8.view(np.uint8),
        })
    return in_maps


def run(x1, x2, trace=False):
    global _cached_nc
    if _cached_nc is None:
        _cached_nc = _build()
    in_maps = _prep_inputs(x1, x2)
    r = bu.run_bass_kernel_spmd(
        _cached_nc, in_maps, core_ids=list(range(NCORES)), trace=trace
    )
    out = np.concatenate([r.results[c]["out"] for c in range(NCORES)], axis=0)
    return out, r


def kernel(x1, x2):
    out, _ = run(x1, x2, trace=False)
    return out


# revision 13
# speedup vs baseline: 8.3386x; 1.0089x over previous
"""L1 pairwise distance kernel for Trainium2, 8 NeuronCores.

res[i, j] = sum_d |x1c[i, d] - x2c[j, d]|,  x1c/x2c centered by mean(x1).

Per core: 256 x1 rows (data-parallel over rows), 2 halves of 128 rows =
64 row-pairs each, split across two producer paths sized so DVE / ACT /
PE finish together:

  A-path (DVE, NA pairs/half): uses |d| = 2*relu(d) - d. One fused
    tensor_scalar (add bias, max 0) per pair emits R = relu(x2c^T - x1)
    in fp16 at 4x mode (~0.66us). One-hot masks with value 2.0 reduce R
    over d into PSUM_A; the remaining -sum_d d = S1[i] - S2[j] is
    rank-1 and lands as one extra matmul per half (lhsT rows: -1 row
    against an S2[j] rhs line, S1[m] row against a ones rhs line).
  J-path (ACT, NJ pairs/half): activation Abs emits |d| tiles directly
    in fp8e4 (ACT cost is dtype-independent). Tile pairs feed DoubleRow
    fp8 matmuls (256-deep contraction, 2 pairs per 512-col chunk) into
    PSUM_J. fp8 rows carry ~1.4e-2 rel err, within the 2e-2 gate.

  PSUM_A copy on DVE, PSUM_J copy on ACT; two DMAs per half write the
  output rows. Input x2 tile is loaded via 4 parallel DMA chunks.

Self-contained: hardcodes shapes from the problem spec.
"""

import numpy as np
import ml_dtypes

import bass_rust
import concourse.bass as bass
import concourse.tile as tile
from concourse import mybir
import concourse.bass_utils as bu

N1 = 2048
N2 = 2048
D = 64
NCORES = 8
IPC = N1 // NCORES          # 256 x1 rows per core
NPAIR_HALF = 64
JCH = 512                   # matmul free-dim chunk (one PSUM bank)
NA = 42                     # DVE relu pairs per half
NJ = NPAIR_HALF - NA        # ACT fp8 pairs per half (even)
NGJ = NJ // 2               # DoubleRow groups (2 pairs each)
F32 = mybir.dt.float32
F16 = mybir.dt.float16
F8 = mybir.dt.float8e4
A = mybir.AluOpType
DR = mybir.MatmulPerfMode.DoubleRow
ABS = mybir.ActivationFunctionType.Abs

_nop_counter = [0]


def _split_multi_waits(nc):
    """This container's walrus build allows one sync-wait per instruction.
    Move extra waits onto same-engine NoOps placed just before."""
    for fn in nc.m.functions:
        for blk in fn.blocks:
            out = []
            changed = False
            for inst in blk.instructions:
                si = inst.sync_info
                if si is not None and len(si.on_wait) > 1:
                    waits = list(si.on_wait)
                    for w in waits[:-1]:
                        _nop_counter[0] += 1
                        nop = mybir.InstNoOp(
                            name=f"I-waitsplit-{_nop_counter[0]}", ins=[], outs=[]
                        )
                        nop.engine = inst.engine
                        nop.sync_info = bass_rust.SyncInfo(on_wait=[w], on_update=[])
                        if inst.debug is not None:
                            nop.debug = inst.debug
                        out.append(nop)
                        nc.register_instruction(nop, overwrite=True)
                    si.on_wait = waits[-1:]
                    changed = True
                out.append(inst)
            if changed:
                blk.instructions = out


def _schedule(counts):
    """Proportional merge: yields (kind, idx) interleaved by fraction."""
    items = []
    for kind, n in counts.items():
        for i in range(n):
            items.append(((i + 0.5) / n, kind, i))
    items.sort()
    return [(k, i) for _, k, i in items]


def _build():
    nc = bass.Bass()
    x2s16_d = nc.dram_tensor("x2s16", [128, N2], F16, kind="ExternalInput")
    bias_d = nc.dram_tensor("bias", [128, IPC // 2], F32, kind="ExternalInput")
    maskb16_d = nc.dram_tensor("maskb16", [128, 254], F16, kind="ExternalInput")
    jmask8_d = nc.dram_tensor("jmask8", [128, NGJ, 2, 128], F8, kind="ExternalInput")
    corrl_d = nc.dram_tensor("corrl", [128, 2, 128], F16, kind="ExternalInput")
    corrr_d = nc.dram_tensor("corrr", [128, N2], F16, kind="ExternalInput")
    out_d = nc.dram_tensor("out", [IPC, N2], F32, kind="ExternalOutput")

    with tile.TileContext(nc) as tc:
        with (
            tc.tile_pool(name="singles", bufs=1) as singles,
            tc.tile_pool(name="ad", bufs=10) as adpool,
            tc.tile_pool(name="jd", bufs=3) as jdpool,
            tc.tile_pool(name="psa", bufs=1, space="PSUM") as psapool,
            tc.tile_pool(name="psj", bufs=1, space="PSUM") as psjpool,
            tc.tile_pool(name="ob", bufs=4) as outpool,
        ):
            x2s16 = singles.tile([128, N2], F16)
            for q in range(4):
                nc.sync.dma_start(
                    x2s16[:, q * 512 : (q + 1) * 512],
                    x2s16_d[:, q * 512 : (q + 1) * 512],
                )
            bias = singles.tile([128, IPC // 2], F32)
            nc.sync.dma_start(bias[:], bias_d[:])
            maskb16 = singles.tile([128, 254], F16)
            nc.sync.dma_start(maskb16[:], maskb16_d[:])
            jmask8 = singles.tile([128, NGJ, 2, 128], F8)
            nc.sync.dma_start(jmask8[:], jmask8_d[:])
            corrl = singles.tile([128, 2, 128], F16)
            nc.sync.dma_start(corrl[:], corrl_d[:])
            corrr = singles.tile([128, N2], F16)
            nc.sync.dma_start(corrr[:], corrr_d[:])

            for h in range(2):
                psA = psapool.tile([128, N2], F32)
                psJ = psjpool.tile([128, N2], F32)
                fp8_done = 0
                for kind, idx in _schedule({"a": NA, "j": NGJ}):
                    if kind == "a":
                        a = idx
                        pi = h * NPAIR_HALF + a
                        ad = adpool.tile([128, N2], F16, tag="ad")
                        nc.vector.tensor_scalar(
                            out=ad[:], in0=x2s16[:],
                            scalar1=bias[:, pi : pi + 1], scalar2=0.0,
                            op0=A.add, op1=A.max,
                        )
                        mg = maskb16[:, 126 - 2 * a : 254 - 2 * a]
                        for jc in range(N2 // JCH):
                            nc.tensor.matmul(
                                psA[:, jc * JCH : (jc + 1) * JCH],
                                mg,
                                ad[:, jc * JCH : (jc + 1) * JCH],
                                start=(a == 0),
                                stop=False,
                            )
                    else:
                        g = idx
                        jt = jdpool.tile([128, 2, N2], F8, tag="jt")
                        for i in range(2):
                            pi = h * NPAIR_HALF + NA + 2 * g + i
                            nc.scalar.activation(
                                out=jt[:, i, :], in_=x2s16[:],
                                func=ABS,
                                bias=bias[:, pi : pi + 1], scale=1.0,
                            )
                        for jc in range(N2 // JCH):
                            nc.tensor.matmul(
                                psJ[:, jc * JCH : (jc + 1) * JCH],
                                jmask8[:, g, :, :],
                                jt[:, :, jc * JCH : (jc + 1) * JCH],
                                start=(fp8_done == 0),
                                stop=(fp8_done == NGJ - 1),
                                perf_mode=DR,
                            )
                        fp8_done += 1

                # rank-1 correction: psA[m,j] += S1[h*128+m] - S2[j] (A rows)
                for jc in range(N2 // JCH):
                    nc.tensor.matmul(
                        psA[:, jc * JCH : (jc + 1) * JCH],
                        corrl[:, h, :],
                        corrr[:, jc * JCH : (jc + 1) * JCH],
                        start=False,
                        stop=True,
                    )

                obA = outpool.tile([128, N2], F32, tag="obA")
                obJ = outpool.tile([128, N2], F32, tag="obJ")
                nc.vector.tensor_copy(obA[:], psA[:])
                nc.scalar.copy(obJ[:], psJ[:])
                nc.sync.dma_start(
                    out_d[h * 128 : h * 128 + 2 * NA, :], obA[0 : 2 * NA, :]
                )
                nc.sync.dma_start(
                    out_d[h * 128 + 2 * NA : (h + 1) * 128, :], obJ[2 * NA : 128, :]
                )
    _split_multi_waits(nc)
    return nc


_cached_nc = None


def _prep_inputs(x1, x2):
    x1 = np.asarray(x1, dtype=np.float32)
    x2 = np.asarray(x2, dtype=np.float32)
    adj = x1.mean(axis=0, dtype=np.float32).astype(np.float32)
    x1c = x1 - adj
    x2c = x2 - adj

    x2s16 = np.tile(np.ascontiguousarray(x2c.T), (2, 1)).astype(np.float16)

    # A-path masks carry the 2x of |d| = 2 relu(d) - d
    maskb = np.zeros((128, 254), dtype=np.float32)
    k = np.arange(128)
    maskb[k, 126 + k // 64] = 2.0
    maskb16 = maskb.astype(np.float16)

    # J: group g covers pairs NA+2g, NA+2g+1 -> partitions 2NA+4g .. +3
    jmask8 = np.zeros((128, NGJ, 2, 128), dtype=ml_dtypes.float8_e4m3fn)
    for g in range(NGJ):
        for i in range(2):
            jmask8[k, g, i, 2 * NA + 4 * g + 2 * i + k // 64] = 1.0

    # rank-1 correction operands (fp16):
    #   corrr row 0 = S2[j], row 1 = 1.0
    #   corrl[0, h, m] = -1, corrl[1, h, m] = S1[h*128+m]   (A rows only)
    S2 = x2c.sum(axis=1, dtype=np.float32)            # [N2]
    corrr = np.zeros((128, N2), dtype=np.float16)
    corrr[0, :] = S2.astype(np.float16)
    corrr[1, :] = 1.0

    in_maps = []
    for c in range(NCORES):
        sl = x1c[c * IPC : (c + 1) * IPC]          # [256, 64]
        b = -np.transpose(sl.reshape(IPC // 2, 2, D), (1, 2, 0)).reshape(128, IPC // 2)
        S1 = sl.sum(axis=1, dtype=np.float32)      # [256]
        corrl = np.zeros((128, 2, 128), dtype=np.float16)
        for h in range(2):
            corrl[0, h, 0 : 2 * NA] = -1.0
            corrl[1, h, 0 : 2 * NA] = S1[h * 128 : h * 128 + 2 * NA].astype(
                np.float16
            )
        in_maps.append({
            "x2s16": x2s16,
            "bias": np.ascontiguousarray(b, dtype=np.float32),
            "maskb16": maskb16,
            "jmask8": jmask8.view(np.uint8),
            "corrl": corrl,
            "corrr": corrr,
        })
    return in_maps


def run(x1, x2, trace=False):
    global _cached_nc
    if _cached_nc is None:
        _cached_nc = _build()
    in_maps = _prep_inputs(x1, x2)
    r = bu.run_bass_kernel_spmd(
        _cached_nc, in_maps, core_ids=list(range(NCORES)), trace=trace
    )
    out = np.concatenate([r.results[c]["out"] for c in range(NCORES)], axis=0)
    return out, r


def kernel(x1, x2):
    out, _ = run(x1, x2, trace=False)
    return out
